# revision 2
# baseline (speedup 1.0000x reference)
"""GAT layer kernel for Trainium2, SPMD over 8 NeuronCores (one batch per core).

Math: the reference's softmax+mask+renorm collapses algebraically —
    softmax(s)*adj * (sum(softmax(s)) / sum(softmax(s)*adj))
  == adj*exp(s) / sum_j(adj*exp(s))           (the softmax denominator cancels)
and exp(leaky_relu(s)) == max(exp(s), exp(0.2*s)) by monotonicity of exp,
so the whole per-batch computation is:
    p      = x[b] @ W.T                               [V, D]
    e_i    = p @ a_left   (row vector over queries i)
    e_j    = p @ a_right  (col vector over keys j)
    st[j,i]= adjT[j,i] * max(exp(e_i+e_j), exp(0.2*(e_i+e_j)))
    outT   = relu( (p_aug.T @ st) row-scaled by 1/den )  where p_aug = [p | 1]
The ones column of p_aug makes the denominator ride the same matmul chain.

Device layout: scores tile st is [j(partition), i(free)]; the big matmul is
  num[d, i] += p_aug[j, d].T @ st[j, i]  accumulated over 16 j-chunks in PSUM,
output is produced transposed [D, V] and flipped on host.
"""

import sys

import numpy as np

sys.path.insert(0, "/opt/trn_rl_repo")

B, V, H, D = 8, 2048, 256, 128
NEG = 0.2
N_CORES = 8
NT = V // 128  # j-chunks of 128 partitions
NQ = V // 512  # i-blocks of 512 (one PSUM bank each)

_cache = {}


def _build():
    from contextlib import ExitStack

    import concourse.bacc as bacc
    import concourse.bass as bass
    import concourse.tile as tile
    from concourse import mybir

    F32 = mybir.dt.float32
    AF = mybir.ActivationFunctionType
    OP = mybir.AluOpType

    nc = bacc.Bacc(
        "TRN2", target_bir_lowering=False, debug=False, num_devices=N_CORES
    )

    xt_d = nc.dram_tensor("xt", [H, V], F32, kind="ExternalInput")
    adj_d = nc.dram_tensor("adjt", [V, V], F32, kind="ExternalInput")
    wg_d = nc.dram_tensor("wg", [H, D + 1], F32, kind="ExternalInput")
    gl_d = nc.dram_tensor("gl", [H, 1], F32, kind="ExternalInput")
    out_d = nc.dram_tensor("outt", [D, V], F32, kind="ExternalOutput")
    ei_d = nc.dram_tensor("ei_scratch", [1, V], F32)
    rc_d = nc.dram_tensor("rc_scratch", [1, V], F32)

    with tile.TileContext(nc) as tc, ExitStack() as ctx:
        const = ctx.enter_context(tc.tile_pool(name="const", bufs=1))
        adjp = ctx.enter_context(tc.tile_pool(name="adjp", bufs=3))
        t1p = ctx.enter_context(tc.tile_pool(name="t1p", bufs=2))
        t2p = ctx.enter_context(tc.tile_pool(name="t2p", bufs=2))
        stp = ctx.enter_context(tc.tile_pool(name="stp", bufs=2))
        otp = ctx.enter_context(tc.tile_pool(name="otp", bufs=3))
        psum = ctx.enter_context(tc.tile_pool(name="psum", bufs=1, space="PSUM"))

        xt_sb = const.tile([128, 2, V], F32, tag="xt")
        wg_sb = const.tile([128, 2, D + 1], F32, tag="wg")
        gl_sb = const.tile([128, 2, 1], F32, tag="gl")
        p_aug = const.tile([128, NT, D + 1], F32, tag="paug")
        ej = const.tile([128, NT], F32, tag="ej")
        ej02 = const.tile([128, NT], F32, tag="ej02")
        ei_row = const.tile([1, V], F32, tag="eirow")
        eib = const.tile([128, V], F32, tag="eib")
        den_r = const.tile([1, V], F32, tag="denr")
        rec_b = const.tile([128, V], F32, tag="recb")

        nc.sync.dma_start(out=xt_sb[:], in_=xt_d.ap().rearrange("(c p) v -> p c v", p=128))
        nc.sync.dma_start(out=wg_sb[:], in_=wg_d.ap().rearrange("(c p) d -> p c d", p=128))
        nc.sync.dma_start(out=gl_sb[:], in_=gl_d.ap().rearrange("(c p) o -> p c o", p=128))

        # ---- Phase A: p_aug [j, D+1] (last col ones), e_j col, e_i row ----
        nc.vector.memset(p_aug[:, :, D : D + 1], 1.0)
        for jt in range(NT):
            ppj = psum.tile([128, D + 1], F32, tag=f"num{jt % 2}")
            for c in range(2):
                nc.tensor.matmul(
                    ppj[:],
                    xt_sb[:, c, jt * 128 : (jt + 1) * 128],
                    wg_sb[:, c, :],
                    start=(c == 0),
                    stop=(c == 1),
                )
            nc.vector.tensor_copy(p_aug[:, jt, 0:D], ppj[:, 0:D])
            nc.vector.tensor_copy(ej[:, jt : jt + 1], ppj[:, D : D + 1])
        nc.vector.tensor_scalar_mul(ej02[:], ej[:], NEG)
        for vc in range(NQ):
            pei = psum.tile([1, 512], F32, tag=f"den{vc % 2}")
            for c in range(2):
                nc.tensor.matmul(
                    pei[:],
                    gl_sb[:, c, :],
                    xt_sb[:, c, vc * 512 : (vc + 1) * 512],
                    start=(c == 0),
                    stop=(c == 1),
                )
            nc.vector.tensor_copy(ei_row[0:1, vc * 512 : (vc + 1) * 512], pei[:])
        # broadcast e_i row across partitions via a DRAM bounce
        nc.sync.dma_start(out=ei_d.ap(), in_=ei_row[:])
        ei_ap = ei_d.ap()
        nc.sync.dma_start(
            out=eib[:],
            in_=bass.AP(tensor=ei_ap.tensor, offset=ei_ap.offset, ap=[[0, 128], [1, V]]),
        )

        # ---- Phase B: accumulate num[d, i] and den[1, i] over j-chunks ----
        nums = [
            psum.tile([128, 512], F32, tag=f"num{q}", name=f"numps{q}")
            for q in range(4)
        ]
        dens = [
            psum.tile([1, 512], F32, tag=f"den{q}", name=f"denps{q}")
            for q in range(4)
        ]
        for jt in range(NT):
            adj_sb = adjp.tile([128, V], F32, tag="adj")
            nc.sync.dma_start(out=adj_sb[:], in_=adj_d[jt * 128 : (jt + 1) * 128, :])
            t1 = t1p.tile([128, V], F32, tag="t1")
            t2 = t2p.tile([128, V], F32, tag="t2")
            st = stp.tile([128, V], F32, tag="st")
            nc.scalar.activation(t1[:], eib[:], AF.Exp, bias=ej[:, jt : jt + 1])
            nc.scalar.activation(t2[:], eib[:], AF.Exp, bias=ej02[:, jt : jt + 1], scale=NEG)
            nc.vector.tensor_max(st[:], t1[:], t2[:])
            nc.vector.tensor_mul(t1[:], st[:], adj_sb[:])
            for q in range(NQ):
                sl = slice(q * 512, (q + 1) * 512)
                nc.tensor.matmul(
                    nums[q][:], p_aug[:, jt, 0:D], t1[:, sl],
                    start=(jt == 0), stop=(jt == NT - 1),
                )
                nc.tensor.matmul(
                    dens[q][:], p_aug[:, jt, D : D + 1], t1[:, sl],
                    start=(jt == 0), stop=(jt == NT - 1),
                )

        # ---- Epilogue: out = relu(num) * (1/den), written transposed ----
        for q in range(NQ):
            nc.vector.reciprocal(den_r[0:1, q * 512 : (q + 1) * 512], dens[q][0:1, :])
        nc.sync.dma_start(out=rc_d.ap(), in_=den_r[:])
        rc_ap = rc_d.ap()
        nc.sync.dma_start(
            out=rec_b[:],
            in_=bass.AP(tensor=rc_ap.tensor, offset=rc_ap.offset, ap=[[0, 128], [1, V]]),
        )
        for q in range(NQ):
            ot = otp.tile([128, 512], F32, tag="ot")
            # relu(num)*rec == relu(num*rec) since rec > 0
            nc.vector.scalar_tensor_tensor(
                ot[:], nums[q][:], 0.0, rec_b[:, q * 512 : (q + 1) * 512],
                op0=mybir.AluOpType.max, op1=mybir.AluOpType.mult,
            )
            nc.sync.dma_start(out=out_d[:, q * 512 : (q + 1) * 512], in_=ot[:])

    nc.compile()
    return nc


def _get_nc():
    if "nc" not in _cache:
        _cache["nc"] = _build()
    return _cache["nc"]


def _prep_in_maps(x, adjacency_matrix, W, a):
    x = np.asarray(x, dtype=np.float32)
    adj = np.asarray(adjacency_matrix)
    W = np.asarray(W, dtype=np.float32)
    a = np.asarray(a, dtype=np.float32)

    adjt = np.ascontiguousarray(adj.T.astype(np.float32))
    wt = np.ascontiguousarray(W.T)  # [H, D]
    gr = wt @ a[0, D:]  # [H]
    gl = (wt @ a[0, :D]).reshape(H, 1).astype(np.float32)
    wg = np.ascontiguousarray(
        np.concatenate([wt, gr.reshape(H, 1)], axis=1)
    ).astype(np.float32)  # [H, D+1]
    xt = np.ascontiguousarray(x.transpose(0, 2, 1))  # [B, H, V]

    return [
        {"xt": xt[c], "adjt": adjt, "wg": wg, "gl": gl}
        for c in range(N_CORES)
    ]


def kernel(x, adjacency_matrix, W, a, trace=False):
    from concourse.bass_utils import run_bass_kernel_spmd

    nc = _get_nc()
    in_maps = _prep_in_maps(x, adjacency_matrix, W, a)
    res = run_bass_kernel_spmd(nc, in_maps, list(range(N_CORES)), trace=trace)
    _cache["last_result"] = res
    out = np.stack([res.results[c]["outt"].T for c in range(N_CORES)], axis=0)
    return np.ascontiguousarray(out.astype(np.float32))


def last_exec_time_ns():
    res = _cache.get("last_result")
    return None if res is None else res.exec_time_ns


# revision 4
# speedup vs baseline: 1.4191x; 1.4191x over previous
"""GAT layer kernel for Trainium2, SPMD over 8 NeuronCores (one batch per core).

Math: the reference's softmax+mask+renorm collapses algebraically —
    softmax(s)*adj * (sum(softmax(s)) / sum(softmax(s)*adj))
  == adj*exp(s) / sum_j(adj*exp(s))           (the softmax denominator cancels)
and exp(leaky_relu(s)) == max(exp(s), exp(0.2*s)) by monotonicity of exp,
so the whole per-batch computation is:
    p      = x[b] @ W.T                               [V, D]
    e_i    = p @ a_left   (row vector over queries i)
    e_j    = p @ a_right  (col vector over keys j)
    st[j,i]= adjT[j,i] * max(exp(e_i+e_j), exp(0.2*(e_i+e_j)))
    outT   = relu( (p_aug.T @ st) row-scaled by 1/den )  where p_aug = [p | 1]
The ones column of p_aug makes the denominator ride the same matmul chain.

Device layout: scores tile st is [j(partition), i(free)]; the big matmul is
  num[d, i] += p_aug[j, d].T @ st[j, i]  accumulated over 16 j-chunks in PSUM,
output is produced transposed [D, V] and flipped on host.
"""

import sys

import numpy as np

sys.path.insert(0, "/opt/trn_rl_repo")

B, V, H, D = 8, 2048, 256, 128
NEG = 0.2
N_CORES = 8
NT = V // 128  # j-chunks of 128 partitions
NQ = V // 512  # i-blocks of 512 (one PSUM bank each)

_cache = {}


def _build():
    from contextlib import ExitStack

    import concourse.bacc as bacc
    import concourse.bass as bass
    import concourse.tile as tile
    from concourse import mybir

    F32 = mybir.dt.float32
    BF16 = mybir.dt.float16
    AF = mybir.ActivationFunctionType
    OP = mybir.AluOpType

    nc = bacc.Bacc(
        "TRN2", target_bir_lowering=False, debug=False, num_devices=N_CORES
    )

    xt_d = nc.dram_tensor("xt", [H, V], F32, kind="ExternalInput")
    adj_d = nc.dram_tensor("adjt", [V, V], BF16, kind="ExternalInput")
    wg_d = nc.dram_tensor("wg", [H, D + 1], F32, kind="ExternalInput")
    gl_d = nc.dram_tensor("gl", [H, 1], F32, kind="ExternalInput")
    out_d = nc.dram_tensor("outt", [D, V], F32, kind="ExternalOutput")
    ei_d = nc.dram_tensor("ei_scratch", [1, V], F32)
    rc_d = nc.dram_tensor("rc_scratch", [1, V], F32)

    with tile.TileContext(nc) as tc, ExitStack() as ctx:
        const = ctx.enter_context(tc.tile_pool(name="const", bufs=1))
        adjp = ctx.enter_context(tc.tile_pool(name="adjp", bufs=3))
        t1p = ctx.enter_context(tc.tile_pool(name="t1p", bufs=2))
        t2p = ctx.enter_context(tc.tile_pool(name="t2p", bufs=2))
        stp = ctx.enter_context(tc.tile_pool(name="stp", bufs=2))
        otp = ctx.enter_context(tc.tile_pool(name="otp", bufs=3))
        psum = ctx.enter_context(tc.tile_pool(name="psum", bufs=1, space="PSUM"))

        xt_sb = const.tile([128, 2, V], F32, tag="xt")
        wg_sb = const.tile([128, 2, D + 1], F32, tag="wg")
        gl_sb = const.tile([128, 2, 1], F32, tag="gl")
        p_aug = const.tile([128, NT, D + 1], BF16, tag="paug")
        ej = const.tile([128, NT], F32, tag="ej")
        ej02 = const.tile([128, NT], F32, tag="ej02")
        ei_row = const.tile([1, V], F32, tag="eirow")
        eib = const.tile([128, V], F32, tag="eib")
        den_r = const.tile([1, V], F32, tag="denr")
        rec_b = const.tile([128, V], F32, tag="recb")

        nc.sync.dma_start(out=xt_sb[:], in_=xt_d.ap().rearrange("(c p) v -> p c v", p=128))
        nc.sync.dma_start(out=wg_sb[:], in_=wg_d.ap().rearrange("(c p) d -> p c d", p=128))
        nc.sync.dma_start(out=gl_sb[:], in_=gl_d.ap().rearrange("(c p) o -> p c o", p=128))

        # ---- Phase A: p_aug [j, D+1] (last col ones), e_j col, e_i row ----
        nc.vector.memset(p_aug[:, :, D : D + 1], 1.0)
        for jt in range(NT):
            ppj = psum.tile([128, D + 1], F32, tag=f"num{jt % 2}")
            for c in range(2):
                nc.tensor.matmul(
                    ppj[:],
                    xt_sb[:, c, jt * 128 : (jt + 1) * 128],
                    wg_sb[:, c, :],
                    start=(c == 0),
                    stop=(c == 1),
                )
            nc.vector.tensor_copy(p_aug[:, jt, 0:D], ppj[:, 0:D])
            nc.vector.tensor_copy(ej[:, jt : jt + 1], ppj[:, D : D + 1])
        nc.vector.tensor_scalar_mul(ej02[:], ej[:], NEG)
        for vc in range(NQ):
            pei = psum.tile([1, 512], F32, tag=f"den{vc % 2}")
            for c in range(2):
                nc.tensor.matmul(
                    pei[:],
                    gl_sb[:, c, :],
                    xt_sb[:, c, vc * 512 : (vc + 1) * 512],
                    start=(c == 0),
                    stop=(c == 1),
                )
            nc.vector.tensor_copy(ei_row[0:1, vc * 512 : (vc + 1) * 512], pei[:])
        # broadcast e_i row across partitions via a DRAM bounce
        nc.sync.dma_start(out=ei_d.ap(), in_=ei_row[:])
        ei_ap = ei_d.ap()
        nc.sync.dma_start(
            out=eib[:],
            in_=bass.AP(tensor=ei_ap.tensor, offset=ei_ap.offset, ap=[[0, 128], [1, V]]),
        )

        # ---- Phase B: accumulate num[d, i] and den[1, i] over j-chunks ----
        nums = [
            psum.tile([128, 512], F32, tag=f"num{q}", name=f"numps{q}")
            for q in range(4)
        ]
        dens = [
            psum.tile([1, 512], F32, tag=f"den{q}", name=f"denps{q}")
            for q in range(4)
        ]
        for jt in range(NT):
            adj_sb = adjp.tile([128, V], BF16, tag="adj")
            nc.sync.dma_start(out=adj_sb[:], in_=adj_d[jt * 128 : (jt + 1) * 128, :])
            t1 = t1p.tile([128, V], BF16, tag="t1")
            t2 = t2p.tile([128, V], BF16, tag="t2")
            st = stp.tile([128, V], BF16, tag="st")
            nc.scalar.activation(t1[:], eib[:], AF.Exp, bias=ej[:, jt : jt + 1])
            nc.scalar.activation(t2[:], eib[:], AF.Exp, bias=ej02[:, jt : jt + 1], scale=NEG)
            nc.vector.tensor_max(st[:], t1[:], t2[:])
            nc.vector.tensor_mul(t1[:], st[:], adj_sb[:])
            for q in range(NQ):
                sl = slice(q * 512, (q + 1) * 512)
                nc.tensor.matmul(
                    nums[q][:], p_aug[:, jt, 0:D], t1[:, sl],
                    start=(jt == 0), stop=(jt == NT - 1),
                )
                nc.tensor.matmul(
                    dens[q][:], p_aug[:, jt, D : D + 1], t1[:, sl],
                    start=(jt == 0), stop=(jt == NT - 1),
                )

        # ---- Epilogue: out = relu(num) * (1/den), written transposed ----
        for q in range(NQ):
            nc.vector.reciprocal(den_r[0:1, q * 512 : (q + 1) * 512], dens[q][0:1, :])
        nc.sync.dma_start(out=rc_d.ap(), in_=den_r[:])
        rc_ap = rc_d.ap()
        nc.sync.dma_start(
            out=rec_b[:],
            in_=bass.AP(tensor=rc_ap.tensor, offset=rc_ap.offset, ap=[[0, 128], [1, V]]),
        )
        for q in range(NQ):
            ot = otp.tile([128, 512], F32, tag="ot")
            # relu(num)*rec == relu(num*rec) since rec > 0
            nc.vector.scalar_tensor_tensor(
                ot[:], nums[q][:], 0.0, rec_b[:, q * 512 : (q + 1) * 512],
                op0=mybir.AluOpType.max, op1=mybir.AluOpType.mult,
            )
            nc.sync.dma_start(out=out_d[:, q * 512 : (q + 1) * 512], in_=ot[:])

    nc.compile()
    return nc


def _get_nc():
    if "nc" not in _cache:
        _cache["nc"] = _build()
    return _cache["nc"]


def _prep_in_maps(x, adjacency_matrix, W, a):
    x = np.asarray(x, dtype=np.float32)
    adj = np.asarray(adjacency_matrix)
    W = np.asarray(W, dtype=np.float32)
    a = np.asarray(a, dtype=np.float32)

    import ml_dtypes
    adjt = np.ascontiguousarray(adj.T.astype(np.float16))
    wt = np.ascontiguousarray(W.T)  # [H, D]
    gr = wt @ a[0, D:]  # [H]
    gl = (wt @ a[0, :D]).reshape(H, 1).astype(np.float32)
    wg = np.ascontiguousarray(
        np.concatenate([wt, gr.reshape(H, 1)], axis=1)
    ).astype(np.float32)  # [H, D+1]
    xt = np.ascontiguousarray(x.transpose(0, 2, 1))  # [B, H, V]

    return [
        {"xt": xt[c], "adjt": adjt, "wg": wg, "gl": gl}
        for c in range(N_CORES)
    ]


def kernel(x, adjacency_matrix, W, a, trace=False):
    from concourse.bass_utils import run_bass_kernel_spmd

    nc = _get_nc()
    in_maps = _prep_in_maps(x, adjacency_matrix, W, a)
    res = run_bass_kernel_spmd(nc, in_maps, list(range(N_CORES)), trace=trace)
    _cache["last_result"] = res
    out = np.stack([res.results[c]["outt"].T for c in range(N_CORES)], axis=0)
    return np.ascontiguousarray(out.astype(np.float32))


def last_exec_time_ns():
    res = _cache.get("last_result")
    return None if res is None else res.exec_time_ns


# revision 5
# speedup vs baseline: 1.4955x; 1.0539x over previous
"""GAT layer kernel for Trainium2, SPMD over 8 NeuronCores (one batch per core).

Math: the reference's softmax+mask+renorm collapses algebraically —
    softmax(s)*adj * (sum(softmax(s)) / sum(softmax(s)*adj))
  == adj*exp(s) / sum_j(adj*exp(s))           (the softmax denominator cancels)
and exp(leaky_relu(s)) == max(exp(s), exp(0.2*s)) by monotonicity of exp,
so the whole per-batch computation is:
    p      = x[b] @ W.T                               [V, D]
    e_i    = p @ a_left   (row vector over queries i)
    e_j    = p @ a_right  (col vector over keys j)
    st[j,i]= adjT[j,i] * max(exp(e_i+e_j), exp(0.2*(e_i+e_j)))
    outT   = relu( (p_aug.T @ st) row-scaled by 1/den )  where p_aug = [p | 1]
The ones column of p_aug makes the denominator ride the same matmul chain.

Device layout: scores tile st is [j(partition), i(free)]; the big matmul is
  num[d, i] += p_aug[j, d].T @ st[j, i]  accumulated over 16 j-chunks in PSUM,
output is produced transposed [D, V] and flipped on host.
"""

import sys

import numpy as np

sys.path.insert(0, "/opt/trn_rl_repo")

B, V, H, D = 8, 2048, 256, 128
NEG = 0.2
N_CORES = 8
NT = V // 128  # j-chunks of 128 partitions
NQ = V // 512  # i-blocks of 512 (one PSUM bank each)

_cache = {}


def _build():
    from contextlib import ExitStack

    import concourse.bacc as bacc
    import concourse.bass as bass
    import concourse.tile as tile
    from concourse import mybir

    F32 = mybir.dt.float32
    BF16 = mybir.dt.float16
    AF = mybir.ActivationFunctionType
    OP = mybir.AluOpType

    nc = bacc.Bacc(
        "TRN2", target_bir_lowering=False, debug=False, num_devices=N_CORES
    )

    xt_d = nc.dram_tensor("xt", [H, V], F32, kind="ExternalInput")
    adj_d = nc.dram_tensor("adjt", [V, V], BF16, kind="ExternalInput")
    wg_d = nc.dram_tensor("wg", [H, D + 1], F32, kind="ExternalInput")
    gl_d = nc.dram_tensor("gl", [H, 1], F32, kind="ExternalInput")
    out_d = nc.dram_tensor("outt", [D, V], F32, kind="ExternalOutput")
    ei_d = nc.dram_tensor("ei_scratch", [1, V], F32)
    rc_d = nc.dram_tensor("rc_scratch", [1, V], F32)

    with tile.TileContext(nc) as tc, ExitStack() as ctx:
        const = ctx.enter_context(tc.tile_pool(name="const", bufs=1))
        adjp = ctx.enter_context(tc.tile_pool(name="adjp", bufs=4))
        t1p = ctx.enter_context(tc.tile_pool(name="t1p", bufs=3))
        t2p = ctx.enter_context(tc.tile_pool(name="t2p", bufs=3))
        smp = ctx.enter_context(tc.tile_pool(name="smp", bufs=2))
        stp = ctx.enter_context(tc.tile_pool(name="stp", bufs=3))
        otp = ctx.enter_context(tc.tile_pool(name="otp", bufs=3))
        psum = ctx.enter_context(tc.tile_pool(name="psum", bufs=1, space="PSUM"))

        xt_sb = const.tile([128, 2, V], F32, tag="xt")
        wg_sb = const.tile([128, 2, D + 1], F32, tag="wg")
        gl_sb = const.tile([128, 2, 1], F32, tag="gl")
        p_aug = const.tile([128, NT, D + 1], BF16, tag="paug")
        ej = const.tile([128, NT], F32, tag="ej")
        ej02 = const.tile([128, NT], F32, tag="ej02")
        ei_row = const.tile([1, V], F32, tag="eirow")
        eib = const.tile([128, V], F32, tag="eib")
        den_r = const.tile([1, V], F32, tag="denr")
        rec_b = const.tile([128, V], F32, tag="recb")

        nc.sync.dma_start(out=xt_sb[:], in_=xt_d.ap().rearrange("(c p) v -> p c v", p=128))
        nc.sync.dma_start(out=wg_sb[:], in_=wg_d.ap().rearrange("(c p) d -> p c d", p=128))
        nc.sync.dma_start(out=gl_sb[:], in_=gl_d.ap().rearrange("(c p) o -> p c o", p=128))

        # ---- Phase A: p_aug [j, D+1] (last col ones), e_j col, e_i row ----
        nc.vector.memset(p_aug[:, :, D : D + 1], 1.0)
        for jt in range(NT):
            ppj = psum.tile([128, D + 1], F32, tag=f"num{jt % 2}")
            for c in range(2):
                nc.tensor.matmul(
                    ppj[:],
                    xt_sb[:, c, jt * 128 : (jt + 1) * 128],
                    wg_sb[:, c, :],
                    start=(c == 0),
                    stop=(c == 1),
                )
            nc.vector.tensor_copy(p_aug[:, jt, 0:D], ppj[:, 0:D])
            nc.vector.tensor_copy(ej[:, jt : jt + 1], ppj[:, D : D + 1])
        nc.vector.tensor_scalar_mul(ej02[:], ej[:], NEG)
        for vc in range(NQ):
            pei = psum.tile([1, 512], F32, tag=f"den{vc % 2}")
            for c in range(2):
                nc.tensor.matmul(
                    pei[:],
                    gl_sb[:, c, :],
                    xt_sb[:, c, vc * 512 : (vc + 1) * 512],
                    start=(c == 0),
                    stop=(c == 1),
                )
            nc.vector.tensor_copy(ei_row[0:1, vc * 512 : (vc + 1) * 512], pei[:])
        # broadcast e_i row across partitions via a DRAM bounce
        nc.sync.dma_start(out=ei_d.ap(), in_=ei_row[:])
        ei_ap = ei_d.ap()
        nc.sync.dma_start(
            out=eib[:],
            in_=bass.AP(tensor=ei_ap.tensor, offset=ei_ap.offset, ap=[[0, 128], [1, V]]),
        )

        # ---- Phase B: accumulate num[d, i] and den[1, i] over j-chunks ----
        nums = [
            psum.tile([128, 512], F32, tag=f"num{q}", name=f"numps{q}")
            for q in range(4)
        ]
        dens = [
            psum.tile([1, 512], F32, tag=f"den{q}", name=f"denps{q}")
            for q in range(4)
        ]
        for jt in range(NT):
            adj_sb = adjp.tile([128, V], BF16, tag="adj")
            nc.sync.dma_start(out=adj_sb[:], in_=adj_d[jt * 128 : (jt + 1) * 128, :])
            t1 = t1p.tile([128, V], BF16, tag="t1")
            t2 = t2p.tile([128, V], BF16, tag="t2")
            sm = smp.tile([128, V], BF16, tag="sm")
            st = stp.tile([128, V], BF16, tag="st")
            nc.scalar.activation(t1[:], eib[:], AF.Exp, bias=ej[:, jt : jt + 1])
            nc.scalar.activation(t2[:], eib[:], AF.Exp, bias=ej02[:, jt : jt + 1], scale=NEG)
            nc.vector.tensor_max(sm[:], t1[:], t2[:])
            nc.vector.tensor_mul(st[:], sm[:], adj_sb[:])
            for q in range(NQ):
                nc.tensor.matmul(
                    nums[q][:], p_aug[:, jt, 0:D], st[:, q * 512 : (q + 1) * 512],
                    start=(jt == 0), stop=(jt == NT - 1),
                )
            for q in range(NQ):
                nc.tensor.matmul(
                    dens[q][:], p_aug[:, jt, D : D + 1], st[:, q * 512 : (q + 1) * 512],
                    start=(jt == 0), stop=(jt == NT - 1),
                )

        # ---- Epilogue: out = relu(num) * (1/den), written transposed ----
        for q in range(NQ):
            nc.vector.reciprocal(den_r[0:1, q * 512 : (q + 1) * 512], dens[q][0:1, :])
        nc.sync.dma_start(out=rc_d.ap(), in_=den_r[:])
        rc_ap = rc_d.ap()
        nc.sync.dma_start(
            out=rec_b[:],
            in_=bass.AP(tensor=rc_ap.tensor, offset=rc_ap.offset, ap=[[0, 128], [1, V]]),
        )
        for q in range(NQ):
            ot = otp.tile([128, 512], F32, tag="ot")
            # relu(num)*rec == relu(num*rec) since rec > 0
            nc.vector.scalar_tensor_tensor(
                ot[:], nums[q][:], 0.0, rec_b[:, q * 512 : (q + 1) * 512],
                op0=mybir.AluOpType.max, op1=mybir.AluOpType.mult,
            )
            nc.sync.dma_start(out=out_d[:, q * 512 : (q + 1) * 512], in_=ot[:])

    nc.compile()
    return nc


def _get_nc():
    if "nc" not in _cache:
        _cache["nc"] = _build()
    return _cache["nc"]


def _prep_in_maps(x, adjacency_matrix, W, a):
    x = np.asarray(x, dtype=np.float32)
    adj = np.asarray(adjacency_matrix)
    W = np.asarray(W, dtype=np.float32)
    a = np.asarray(a, dtype=np.float32)

    import ml_dtypes
    adjt = np.ascontiguousarray(adj.T.astype(np.float16))
    wt = np.ascontiguousarray(W.T)  # [H, D]
    gr = wt @ a[0, D:]  # [H]
    gl = (wt @ a[0, :D]).reshape(H, 1).astype(np.float32)
    wg = np.ascontiguousarray(
        np.concatenate([wt, gr.reshape(H, 1)], axis=1)
    ).astype(np.float32)  # [H, D+1]
    xt = np.ascontiguousarray(x.transpose(0, 2, 1))  # [B, H, V]

    return [
        {"xt": xt[c], "adjt": adjt, "wg": wg, "gl": gl}
        for c in range(N_CORES)
    ]


def kernel(x, adjacency_matrix, W, a, trace=False):
    from concourse.bass_utils import run_bass_kernel_spmd

    nc = _get_nc()
    in_maps = _prep_in_maps(x, adjacency_matrix, W, a)
    res = run_bass_kernel_spmd(nc, in_maps, list(range(N_CORES)), trace=trace)
    _cache["last_result"] = res
    out = np.stack([res.results[c]["outt"].T for c in range(N_CORES)], axis=0)
    return np.ascontiguousarray(out.astype(np.float32))


def last_exec_time_ns():
    res = _cache.get("last_result")
    return None if res is None else res.exec_time_ns


# revision 6
# speedup vs baseline: 1.5781x; 1.0552x over previous
"""GAT layer kernel for Trainium2, SPMD over 8 NeuronCores (one batch per core).

Math: the reference's softmax+mask+renorm collapses algebraically —
    softmax(s)*adj * (sum(softmax(s)) / sum(softmax(s)*adj))
  == adj*exp(s) / sum_j(adj*exp(s))           (the softmax denominator cancels)
and exp(leaky_relu(s)) == max(exp(s), exp(0.2*s)) by monotonicity of exp,
so the whole per-batch computation is:
    p      = x[b] @ W.T                               [V, D]
    e_i    = p @ a_left   (row vector over queries i)
    e_j    = p @ a_right  (col vector over keys j)
    st[j,i]= adjT[j,i] * max(exp(e_i+e_j), exp(0.2*(e_i+e_j)))
    outT   = relu( (p_aug.T @ st) row-scaled by 1/den )  where p_aug = [p | 1]
The ones column of p_aug makes the denominator ride the same matmul chain.

Device layout: scores tile st is [j(partition), i(free)]; the big matmul is
  num[d, i] += p_aug[j, d].T @ st[j, i]  accumulated over 16 j-chunks in PSUM,
output is produced transposed [D, V] and flipped on host.
"""

import sys

import numpy as np

sys.path.insert(0, "/opt/trn_rl_repo")

B, V, H, D = 8, 2048, 256, 128
NEG = 0.2
N_CORES = 8
NT = V // 128  # j-chunks of 128 partitions
NQ = V // 512  # i-blocks of 512 (one PSUM bank each)

_cache = {}


def _build():
    from contextlib import ExitStack

    import concourse.bacc as bacc
    import concourse.bass as bass
    import concourse.tile as tile
    from concourse import mybir

    F32 = mybir.dt.float32
    BF16 = mybir.dt.float16
    AF = mybir.ActivationFunctionType
    OP = mybir.AluOpType

    nc = bacc.Bacc(
        "TRN2", target_bir_lowering=False, debug=False, num_devices=N_CORES
    )

    xt_d = nc.dram_tensor("xt", [H, V], F32, kind="ExternalInput")
    adj_d = nc.dram_tensor("adjt", [V, V], BF16, kind="ExternalInput")
    wg_d = nc.dram_tensor("wg", [H, D + 1], F32, kind="ExternalInput")
    gl_d = nc.dram_tensor("gl", [H, 1], F32, kind="ExternalInput")
    out_d = nc.dram_tensor("outt", [D, V], F32, kind="ExternalOutput")
    ei_d = nc.dram_tensor("ei_scratch", [1, V], F32)
    rc_d = nc.dram_tensor("rc_scratch", [1, V], F32)

    with tile.TileContext(nc) as tc, ExitStack() as ctx:
        const = ctx.enter_context(tc.tile_pool(name="const", bufs=1))
        adjp = ctx.enter_context(tc.tile_pool(name="adjp", bufs=6))
        t1p = ctx.enter_context(tc.tile_pool(name="t1p", bufs=3))
        t2p = ctx.enter_context(tc.tile_pool(name="t2p", bufs=2))
        smp = ctx.enter_context(tc.tile_pool(name="smp", bufs=2))
        sfp = ctx.enter_context(tc.tile_pool(name="sfp", bufs=2))
        spp = ctx.enter_context(tc.tile_pool(name="spp", bufs=2))
        stp = ctx.enter_context(tc.tile_pool(name="stp", bufs=6))
        otp = ctx.enter_context(tc.tile_pool(name="otp", bufs=3))
        psum = ctx.enter_context(tc.tile_pool(name="psum", bufs=1, space="PSUM"))

        xt_sb = const.tile([128, 2, V], F32, tag="xt")
        wg_sb = const.tile([128, 2, D + 1], F32, tag="wg")
        gl_sb = const.tile([128, 2, 1], F32, tag="gl")
        p_aug = const.tile([128, NT, D + 1], BF16, tag="paug")
        ej = const.tile([128, NT], F32, tag="ej")
        ej02 = const.tile([128, NT], F32, tag="ej02")
        ei_row = const.tile([1, V], F32, tag="eirow")
        eib = const.tile([128, V], F32, tag="eib")
        den_r = const.tile([1, V], F32, tag="denr")
        rec_b = const.tile([128, V], F32, tag="recb")

        nc.sync.dma_start(out=xt_sb[:], in_=xt_d.ap().rearrange("(c p) v -> p c v", p=128))
        nc.sync.dma_start(out=wg_sb[:], in_=wg_d.ap().rearrange("(c p) d -> p c d", p=128))
        nc.sync.dma_start(out=gl_sb[:], in_=gl_d.ap().rearrange("(c p) o -> p c o", p=128))

        # ---- Phase A: p_aug [j, D+1] (last col ones), e_j col, e_i row ----
        nc.vector.memset(p_aug[:, :, D : D + 1], 1.0)
        for jt in range(NT):
            ppj = psum.tile([128, D + 1], F32, tag=f"num{jt % 2}")
            for c in range(2):
                nc.tensor.matmul(
                    ppj[:],
                    xt_sb[:, c, jt * 128 : (jt + 1) * 128],
                    wg_sb[:, c, :],
                    start=(c == 0),
                    stop=(c == 1),
                )
            nc.scalar.copy(p_aug[:, jt, 0:D], ppj[:, 0:D])
            nc.scalar.copy(ej[:, jt : jt + 1], ppj[:, D : D + 1])
        nc.vector.tensor_scalar_mul(ej02[:], ej[:], NEG)
        for vc in range(NQ):
            pei = psum.tile([1, 512], F32, tag=f"den{vc % 2}")
            for c in range(2):
                nc.tensor.matmul(
                    pei[:],
                    gl_sb[:, c, :],
                    xt_sb[:, c, vc * 512 : (vc + 1) * 512],
                    start=(c == 0),
                    stop=(c == 1),
                )
            nc.scalar.copy(ei_row[0:1, vc * 512 : (vc + 1) * 512], pei[:])
        # broadcast e_i row across partitions via a DRAM bounce
        nc.sync.dma_start(out=ei_d.ap(), in_=ei_row[:])
        ei_ap = ei_d.ap()
        nc.sync.dma_start(
            out=eib[:],
            in_=bass.AP(tensor=ei_ap.tensor, offset=ei_ap.offset, ap=[[0, 128], [1, V]]),
        )

        # ---- Phase B: accumulate num[d, i] and den[1, i] over j-chunks ----
        nums = [
            psum.tile([128, 512], F32, tag=f"num{q}", name=f"numps{q}")
            for q in range(4)
        ]
        dens = [
            psum.tile([1, 512], F32, tag=f"den{q}", name=f"denps{q}")
            for q in range(4)
        ]
        def make_st(jt, use_act_path):
            adj_sb = adjp.tile([128, V], BF16, tag="adj", name=f"adj{jt}")
            nc.sync.dma_start(out=adj_sb[:], in_=adj_d[jt * 128 : (jt + 1) * 128, :])
            st = stp.tile([128, V], BF16, tag="st", name=f"st{jt}")
            if use_act_path:
                # 2 ACT passes + 1 DVE mul:  st = adj * max(exp(s), exp(0.2 s))
                t1 = t1p.tile([128, V], BF16, tag="t1", name=f"t1_{jt}")
                t2 = t2p.tile([128, V], BF16, tag="t2", name=f"t2_{jt}")
                sm = smp.tile([128, V], BF16, tag="sm", name=f"sm{jt}")
                nc.scalar.activation(t1[:], eib[:], AF.Exp, bias=ej[:, jt : jt + 1])
                nc.scalar.activation(
                    t2[:], eib[:], AF.Exp, bias=ej02[:, jt : jt + 1], scale=NEG
                )
                nc.vector.tensor_max(sm[:], t1[:], t2[:])
                nc.vector.tensor_mul(st[:], sm[:], adj_sb[:])
            else:
                # LR on DVE + 1 ACT pass:  st = adj * exp(max(s, 0.2 s))
                sf = sfp.tile([128, V], BF16, tag="sf", name=f"sf{jt}")
                sp = spp.tile([128, V], BF16, tag="sp", name=f"sp{jt}")
                t1 = t1p.tile([128, V], BF16, tag="t1", name=f"t1_{jt}")
                nc.vector.tensor_scalar_add(sf[:], eib[:], ej[:, jt : jt + 1])
                nc.vector.scalar_tensor_tensor(
                    sp[:], sf[:], NEG, sf[:], op0=OP.mult, op1=OP.max
                )
                nc.scalar.activation(t1[:], sp[:], AF.Exp)
                nc.vector.tensor_mul(st[:], t1[:], adj_sb[:])
            return st

        GRP = 4
        for g in range(NT // GRP):
            sts = []
            for k in range(GRP):
                jt = g * GRP + k
                sts.append((jt, make_st(jt, use_act_path=(k % 2 == 0))))
            for jt, st in sts:
                for q in range(NQ):
                    nc.tensor.matmul(
                        nums[q][:], p_aug[:, jt, 0:D], st[:, q * 512 : (q + 1) * 512],
                        start=(jt == 0), stop=(jt == NT - 1),
                    )
                for q in range(NQ):
                    nc.tensor.matmul(
                        dens[q][:], p_aug[:, jt, D : D + 1], st[:, q * 512 : (q + 1) * 512],
                        start=(jt == 0), stop=(jt == NT - 1),
                    )

        # ---- Epilogue: out = relu(num) / den, written transposed ----
        for q in range(NQ):
            nc.scalar.copy(den_r[0:1, q * 512 : (q + 1) * 512], dens[q][0:1, :])
        nc.sync.dma_start(out=rc_d.ap(), in_=den_r[:])
        rc_ap = rc_d.ap()
        nc.sync.dma_start(
            out=rec_b[:],
            in_=bass.AP(tensor=rc_ap.tensor, offset=rc_ap.offset, ap=[[0, 128], [1, V]]),
        )
        nc.vector.reciprocal(rec_b[:], rec_b[:])
        for q in range(NQ):
            ot = otp.tile([128, 512], F32, tag="ot", name=f"ot{q}")
            # relu(num)*rec == relu(num*rec) since rec > 0
            nc.vector.scalar_tensor_tensor(
                ot[:], nums[q][:], 0.0, rec_b[:, q * 512 : (q + 1) * 512],
                op0=mybir.AluOpType.max, op1=mybir.AluOpType.mult,
            )
            nc.sync.dma_start(out=out_d[:, q * 512 : (q + 1) * 512], in_=ot[:])

    nc.compile()
    return nc


def _get_nc():
    if "nc" not in _cache:
        _cache["nc"] = _build()
    return _cache["nc"]


def _prep_in_maps(x, adjacency_matrix, W, a):
    x = np.asarray(x, dtype=np.float32)
    adj = np.asarray(adjacency_matrix)
    W = np.asarray(W, dtype=np.float32)
    a = np.asarray(a, dtype=np.float32)

    import ml_dtypes
    adjt = np.ascontiguousarray(adj.T.astype(np.float16))
    wt = np.ascontiguousarray(W.T)  # [H, D]
    gr = wt @ a[0, D:]  # [H]
    gl = (wt @ a[0, :D]).reshape(H, 1).astype(np.float32)
    wg = np.ascontiguousarray(
        np.concatenate([wt, gr.reshape(H, 1)], axis=1)
    ).astype(np.float32)  # [H, D+1]
    xt = np.ascontiguousarray(x.transpose(0, 2, 1))  # [B, H, V]

    return [
        {"xt": xt[c], "adjt": adjt, "wg": wg, "gl": gl}
        for c in range(N_CORES)
    ]


def kernel(x, adjacency_matrix, W, a, trace=False):
    from concourse.bass_utils import run_bass_kernel_spmd

    nc = _get_nc()
    in_maps = _prep_in_maps(x, adjacency_matrix, W, a)
    res = run_bass_kernel_spmd(nc, in_maps, list(range(N_CORES)), trace=trace)
    _cache["last_result"] = res
    out = np.stack([res.results[c]["outt"].T for c in range(N_CORES)], axis=0)
    return np.ascontiguousarray(out.astype(np.float32))


def last_exec_time_ns():
    res = _cache.get("last_result")
    return None if res is None else res.exec_time_ns


# revision 8
# speedup vs baseline: 1.7797x; 1.1277x over previous
"""GAT layer kernel for Trainium2, SPMD over 8 NeuronCores (one batch per core).

Math: the reference's softmax+mask+renorm collapses algebraically —
    softmax(s)*adj * (sum(softmax(s)) / sum(softmax(s)*adj))
  == adj*exp(s) / sum_j(adj*exp(s))           (the softmax denominator cancels)
and exp(leaky_relu(s)) == max(exp(s), exp(0.2*s)) == exp(max(s, 0.2*s)),
so the whole per-batch computation is:
    p      = x[b] @ W.T                               [V, D]
    e_i    = p @ a_left   (row vector over queries i)
    e_j    = p @ a_right  (col vector over keys j)
    st[j,i]= adjT[j,i] * exp(LR(e_i+e_j))
    outT   = relu( (p_aug.T @ st) col-scaled by 1/den )  where p_aug = [p | 1]
The ones column of p_aug makes the denominator ride the same matmul chain.

Device layout: scores tile st is [j(partition), i(free)]; the big matmul is
  num[d, i] += p_aug[j, d].T @ st[j, i]  accumulated over 16 j-chunks in PSUM,
output is produced transposed [D, V] and flipped on host.

Engine balance per j-chunk (alternating):
  - ACT-path: t1=exp(s), t2=exp(0.2s) on ACT; max+mask on DVE
  - DVE-path: s=add, LR=scalar_tensor_tensor on DVE; one exp on ACT; mask on DVE
Matmul operands are bf16 (PE fast path; fp16 streams at half rate).
"""

import sys

import numpy as np

sys.path.insert(0, "/opt/trn_rl_repo")

B, V, H, D = 8, 2048, 256, 128
NEG = 0.2
N_CORES = 8
NT = V // 128  # j-chunks of 128 partitions
NQ = V // 512  # i-blocks of 512 (one PSUM bank each)

_cache = {}


def _build():
    from contextlib import ExitStack

    import concourse.bacc as bacc
    import concourse.bass as bass
    import concourse.tile as tile
    from concourse import mybir

    F32 = mybir.dt.float32
    BF16 = mybir.dt.bfloat16
    AF = mybir.ActivationFunctionType
    OP = mybir.AluOpType

    nc = bacc.Bacc(
        "TRN2", target_bir_lowering=False, debug=False, num_devices=N_CORES
    )

    xt_d = nc.dram_tensor("xt", [H, V], F32, kind="ExternalInput")
    adj_d = nc.dram_tensor("adjt", [V, V], BF16, kind="ExternalInput")
    wg_d = nc.dram_tensor("wg", [H, D + 1], F32, kind="ExternalInput")
    gl_d = nc.dram_tensor("gl", [H, 1], F32, kind="ExternalInput")
    out_d = nc.dram_tensor("outt", [D, V], F32, kind="ExternalOutput")
    ei_d = nc.dram_tensor("ei_scratch", [1, V], F32)

    with tile.TileContext(nc) as tc, ExitStack() as ctx:
        const = ctx.enter_context(tc.tile_pool(name="const", bufs=1))
        adjp = ctx.enter_context(tc.tile_pool(name="adjp", bufs=6))
        t1p = ctx.enter_context(tc.tile_pool(name="t1p", bufs=3))
        t2p = ctx.enter_context(tc.tile_pool(name="t2p", bufs=2))
        smp = ctx.enter_context(tc.tile_pool(name="smp", bufs=2))
        sfp = ctx.enter_context(tc.tile_pool(name="sfp", bufs=2))
        spp = ctx.enter_context(tc.tile_pool(name="spp", bufs=2))
        stp = ctx.enter_context(tc.tile_pool(name="stp", bufs=6))
        otp = ctx.enter_context(tc.tile_pool(name="otp", bufs=4))
        psum = ctx.enter_context(tc.tile_pool(name="psum", bufs=1, space="PSUM"))

        xt_sb = const.tile([128, 2, V], F32, tag="xt")
        wg_sb = const.tile([128, 2, D + 1], F32, tag="wg")
        gl_sb = const.tile([128, 2, 1], F32, tag="gl")
        p_aug = const.tile([128, NT, D + 1], BF16, tag="paug")
        ej = const.tile([128, NT], F32, tag="ej")
        ej02 = const.tile([128, NT], F32, tag="ej02")
        ei_row = const.tile([1, V], F32, tag="eirow")
        eib = const.tile([128, V], F32, tag="eib")
        den_r = const.tile([1, V], F32, tag="denr")
        ones_r = const.tile([1, 128], F32, tag="onesr")

        nc.sync.dma_start(out=wg_sb[:], in_=wg_d.ap().rearrange("(c p) d -> p c d", p=128))
        nc.sync.dma_start(out=gl_sb[:], in_=gl_d.ap().rearrange("(c p) o -> p c o", p=128))
        nc.vector.memset(ones_r[:], 1.0)
        nc.vector.memset(p_aug[:, :, D : D + 1], 1.0)

        xt_ap = xt_d.ap().rearrange("(c p) v -> p c v", p=128)
        for vc in range(NQ):
            nc.sync.dma_start(
                out=xt_sb[:, :, vc * 512 : (vc + 1) * 512],
                in_=xt_ap[:, :, vc * 512 : (vc + 1) * 512],
            )

        # ---- Phase A: p_aug [j, D+1] (last col ones), e_j col, e_i row ----
        for vc in range(NQ):
            pei = psum.tile([1, 512], F32, tag=f"den{vc}", name=f"peips{vc}")
            for c in range(2):
                nc.tensor.matmul(
                    pei[:],
                    gl_sb[:, c, :],
                    xt_sb[:, c, vc * 512 : (vc + 1) * 512],
                    start=(c == 0),
                    stop=(c == 1),
                )
            nc.scalar.copy(ei_row[0:1, vc * 512 : (vc + 1) * 512], pei[:])
            for k in range(4):
                jt = vc * 4 + k
                ppj = psum.tile([128, D + 1], F32, tag=f"num{jt % 4}", name=f"ppjps{jt}")
                for c in range(2):
                    nc.tensor.matmul(
                        ppj[:],
                        xt_sb[:, c, jt * 128 : (jt + 1) * 128],
                        wg_sb[:, c, :],
                        start=(c == 0),
                        stop=(c == 1),
                    )
                nc.scalar.copy(p_aug[:, jt, 0:D], ppj[:, 0:D])
                nc.scalar.copy(ej[:, jt : jt + 1], ppj[:, D : D + 1])
        nc.vector.tensor_scalar_mul(ej02[:], ej[:], NEG)
        # broadcast e_i row across partitions via a DRAM bounce
        nc.sync.dma_start(out=ei_d.ap(), in_=ei_row[:])
        ei_ap = ei_d.ap()
        nc.sync.dma_start(
            out=eib[:],
            in_=bass.AP(tensor=ei_ap.tensor, offset=ei_ap.offset, ap=[[0, 128], [1, V]]),
        )

        # ---- Phase B: accumulate num[d, i] and den[1, i] over j-chunks ----
        nums = [
            psum.tile([128, 512], F32, tag=f"num{q}", name=f"numps{q}")
            for q in range(4)
        ]
        dens = [
            psum.tile([1, 512], F32, tag=f"den{q}", name=f"denps{q}")
            for q in range(4)
        ]

        def make_st(jt, use_act_path):
            adj_sb = adjp.tile([128, V], BF16, tag="adj", name=f"adj{jt}")
            nc.sync.dma_start(out=adj_sb[:], in_=adj_d[jt * 128 : (jt + 1) * 128, :])
            st = stp.tile([128, V], BF16, tag="st", name=f"st{jt}")
            if use_act_path:
                # 2 ACT passes + 1 DVE mul:  st = adj * max(exp(s), exp(0.2 s))
                t1 = t1p.tile([128, V], BF16, tag="t1", name=f"t1_{jt}")
                t2 = t2p.tile([128, V], BF16, tag="t2", name=f"t2_{jt}")
                sm = smp.tile([128, V], BF16, tag="sm", name=f"sm{jt}")
                nc.scalar.activation(t1[:], eib[:], AF.Exp, bias=ej[:, jt : jt + 1])
                nc.scalar.activation(
                    t2[:], eib[:], AF.Exp, bias=ej02[:, jt : jt + 1], scale=NEG
                )
                nc.vector.tensor_max(sm[:], t1[:], t2[:])
                nc.vector.tensor_mul(st[:], sm[:], adj_sb[:])
            else:
                # LR on DVE + 1 ACT pass:  st = adj * exp(max(s, 0.2 s))
                sf = sfp.tile([128, V], BF16, tag="sf", name=f"sf{jt}")
                sp = spp.tile([128, V], BF16, tag="sp", name=f"sp{jt}")
                t1 = t1p.tile([128, V], BF16, tag="t1", name=f"t1_{jt}")
                nc.vector.tensor_scalar_add(sf[:], eib[:], ej[:, jt : jt + 1])
                nc.vector.scalar_tensor_tensor(
                    sp[:], sf[:], NEG, sf[:], op0=OP.mult, op1=OP.max
                )
                nc.scalar.activation(t1[:], sp[:], AF.Exp)
                nc.vector.tensor_mul(st[:], t1[:], adj_sb[:])
            return st

        GRP = 4
        for g in range(NT // GRP):
            sts = []
            for k in range(GRP):
                jt = g * GRP + k
                sts.append((jt, make_st(jt, use_act_path=(k % 2 == 0))))
            for jt, st in sts:
                for q in range(NQ):
                    nc.tensor.matmul(
                        nums[q][:], p_aug[:, jt, 0:D], st[:, q * 512 : (q + 1) * 512],
                        start=(jt == 0), stop=(jt == NT - 1),
                    )
                for q in range(NQ):
                    nc.tensor.matmul(
                        dens[q][:], p_aug[:, jt, D : D + 1], st[:, q * 512 : (q + 1) * 512],
                        start=(jt == 0), stop=(jt == NT - 1),
                    )

        # ---- Epilogue: out = relu(num) / den, written transposed ----
        # 1/den on the (by now idle) ACT engine — one table switch — then
        # broadcast across partitions with a K=1 matmul and multiply via STT.
        for q in range(NQ):
            nc.scalar.copy(den_r[0:1, q * 512 : (q + 1) * 512], dens[q][0:1, :])
        # 1/den = exp(-ln(den)) — Ln+Exp share one ACT table set, and ACT
        # Reciprocal is blocked for accuracy. den ∈ [~200, ~6000], well inside
        # both functions' accurate range.
        nc.scalar.activation(den_r[:], den_r[:], AF.Ln)
        nc.scalar.activation(den_r[:], den_r[:], AF.Exp, scale=-1.0)
        for q in range(NQ):
            rec_ps = psum.tile([128, 512], F32, tag=f"den{q}", name=f"recps{q}")
            nc.tensor.matmul(
                rec_ps[:], ones_r[:], den_r[0:1, q * 512 : (q + 1) * 512],
                start=True, stop=True,
            )
            rec_sb = otp.tile([128, 512], F32, tag="rec", name=f"recsb{q}")
            nc.scalar.copy(rec_sb[:], rec_ps[:])
            ot = otp.tile([128, 512], F32, tag="ot", name=f"ot{q}")
            # relu(num)*rec == relu(num*rec) since rec > 0
            nc.vector.scalar_tensor_tensor(
                ot[:], nums[q][:], 0.0, rec_sb[:],
                op0=OP.max, op1=OP.mult,
            )
            nc.sync.dma_start(out=out_d[:, q * 512 : (q + 1) * 512], in_=ot[:])

    nc.compile()
    return nc


def _get_nc():
    if "nc" not in _cache:
        _cache["nc"] = _build()
    return _cache["nc"]


def _prep_in_maps(x, adjacency_matrix, W, a):
    import ml_dtypes

    x = np.asarray(x, dtype=np.float32)
    adj = np.asarray(adjacency_matrix)
    W = np.asarray(W, dtype=np.float32)
    a = np.asarray(a, dtype=np.float32)

    adjt = np.ascontiguousarray(adj.T.astype(ml_dtypes.bfloat16))
    wt = np.ascontiguousarray(W.T)  # [H, D]
    gr = wt @ a[0, D:]  # [H]
    gl = (wt @ a[0, :D]).reshape(H, 1).astype(np.float32)
    wg = np.ascontiguousarray(
        np.concatenate([wt, gr.reshape(H, 1)], axis=1)
    ).astype(np.float32)  # [H, D+1]
    xt = np.ascontiguousarray(x.transpose(0, 2, 1))  # [B, H, V]

    return [
        {"xt": xt[c], "adjt": adjt, "wg": wg, "gl": gl}
        for c in range(N_CORES)
    ]


def kernel(x, adjacency_matrix, W, a, trace=False):
    from concourse.bass_utils import run_bass_kernel_spmd

    nc = _get_nc()
    in_maps = _prep_in_maps(x, adjacency_matrix, W, a)
    res = run_bass_kernel_spmd(nc, in_maps, list(range(N_CORES)), trace=trace)
    _cache["last_result"] = res
    out = np.stack([res.results[c]["outt"].T for c in range(N_CORES)], axis=0)
    return np.ascontiguousarray(out.astype(np.float32))


def last_exec_time_ns():
    res = _cache.get("last_result")
    return None if res is None else res.exec_time_ns
